# revision 17
# baseline (speedup 1.0000x reference)
"""MultiHeadAttention Trainium2 kernel (8-core SPMD, head/tensor parallel).

Problem (hardcoded shapes): stream (2048, 2, 1024) f32, mask (1, 2048, 2048),
w_qkv (1024, 3072), b_qkv (3072,), w_out (1024, 1024), b_out (1024,).
N=2048, B=2, HEADS=16, D_KQ=D_V=64, D_HEAD=192.

Sharding: core d handles batch b=d//4 and 4 heads [4*(d%4), 4*(d%4)+4).
Per-core compute, all in the "transposed" orientation (no on-device transposes;
the host pre-transposes stream and mask during sharding):

  qkT[f, n]   = (w_qkv_local.T @ x_b.T)[f, n] + b     (f = head-pair d dims)
  v[m, dv]    = (x_b @ w_v_local)[m, dv] + b_v
  logitsT     = k qT  per head:  lT[m, n] = sum_d k[m,d] q[n,d]
  wT[m, n]    = exp(lT) * exp(maskT)[m, n]            (softmax unnormalized)
  psv         = [v | 1]^T-style PV matmul giving values^T rows + replicated
                row-sums Z (the softmax denominator) for free
  valT[hv, n] = psv_values * (1/Z)                    (per-n normalization)
  out_partial = valT^T @ w_out_local                  (summed on host per batch)

Matmuls run as float32r (full-rate fp32 storage) except PV which is bf16
(required so the mask multiply runs at DVE 2x bf16 rate).
"""

import numpy as np
import ml_dtypes

import concourse.bass as bass
import concourse.tile as tile
from concourse import bacc, mybir
from concourse.bass_utils import run_bass_kernel_spmd

BF16 = ml_dtypes.bfloat16
dt = mybir.dt
AF = mybir.ActivationFunctionType

# Shapes (hardcoded per the problem spec)
N = 2048          # sequence length
B = 2             # batch
DSTR = 1024       # d_stream
HEADS = 16        # total heads
NH = 4            # heads per core
DKQ = 64
DV = 64
DHEAD = 2 * DKQ + DV
P = 128
KT = DSTR // P    # 8 contraction k-tiles for projections
MT = N // P       # 16 m-tiles
CH = 1024         # attention n-chunk width
NCH = N // CH     # 2 chunks
NB = 512          # matmul moving free dim
N_CORES = 8

f32, f32r, bf16 = dt.float32, dt.float32r, dt.bfloat16

_BUILT = {}


def _build_nc():
    """Build + compile the single-core SPMD Bass program (same on all cores)."""
    nc = bacc.Bacc("TRN2", target_bir_lowering=False, debug=False)

    xT = nc.dram_tensor("xT", [DSTR, N], f32r, kind="ExternalInput").ap()
    wqk = nc.dram_tensor("wqk", [DSTR, 4 * P], f32r, kind="ExternalInput").ap()
    wv = nc.dram_tensor("wv", [DSTR, NH * DV], f32r, kind="ExternalInput").ap()
    bqk = nc.dram_tensor("bqk", [P, 4], f32, kind="ExternalInput").ap()
    bv = nc.dram_tensor("bv", [1, NH * DV], f32r, kind="ExternalInput").ap()
    ones = nc.dram_tensor("ones", [1, P], f32r, kind="ExternalInput").ap()
    emT = nc.dram_tensor("emT", [N, N], bf16, kind="ExternalInput").ap()
    wout = nc.dram_tensor("wout", [NH * DV, DSTR], f32r, kind="ExternalInput").ap()
    out = nc.dram_tensor("out", [N, DSTR], f32, kind="ExternalOutput").ap()

    with tile.TileContext(nc) as tc:
        with (
            tc.tile_pool(name="consts", bufs=1) as consts,
            tc.tile_pool(name="xw", bufs=1) as xw_p,
            tc.tile_pool(name="qkT", bufs=1) as qkT_p,
            tc.tile_pool(name="v1", bufs=1) as v1_p,
            tc.tile_pool(name="valT", bufs=1) as valT_p,
            tc.tile_pool(name="mask", bufs=2) as mask_p,
            tc.tile_pool(name="wT", bufs=2) as wT_p,
            tc.tile_pool(name="z", bufs=1) as z_p,
            tc.tile_pool(name="ps", bufs=1, space="PSUM") as ps_p,
        ):
            # ---------- persistent SBUF ----------
            bqk_sb = consts.tile([P, 4], f32)
            nc.gpsimd.dma_start(out=bqk_sb, in_=bqk)
            bv_sb = consts.tile([1, NH * DV], f32r)
            nc.gpsimd.dma_start(out=bv_sb, in_=bv)
            ones1 = consts.tile([1, P], f32r)
            nc.gpsimd.dma_start(out=ones1, in_=ones)
            wout_sb = consts.tile([P, 2, DSTR], f32r)
            nc.gpsimd.dma_start(
                out=wout_sb, in_=wout.rearrange("(i p) d -> p i d", p=P))

            xT_sb = xw_p.tile([P, KT, N], f32r)
            wqk_sb = xw_p.tile([P, KT, 4 * P], f32r)
            wv_sb = xw_p.tile([P, KT, NH * DV], f32r)
            nc.gpsimd.dma_start(
                out=wqk_sb, in_=wqk.rearrange("(kt p) f -> p kt f", p=P))
            nc.gpsimd.dma_start(
                out=wv_sb, in_=wv.rearrange("(kt p) f -> p kt f", p=P))
            for g in range(KT // 2):
                nc.sync.dma_start(
                    out=xT_sb[:, 2 * g:2 * g + 2, :],
                    in_=xT[g * 2 * P:(g + 1) * 2 * P, :].rearrange(
                        "(kt p) n -> p kt n", p=P))

            # qkT f-tiles: 0 = q pair0, 1 = q pair1, 2 = k pair0, 3 = k pair1
            # (within a tile: partitions 0:64 = even head's d, 64:128 = odd head's)
            qkT = qkT_p.tile([P, 4, N], f32r)
            # v1 lhsT slots per (mt, head): even-in-pair = [v | ones],
            # odd-in-pair = [ones | v] -> PV output carries values rows and
            # 64x-replicated Z rows in complementary partition halves.
            v1 = v1_p.tile([P, MT, NH, P], bf16)
            nc.vector.memset(v1, 0.0)
            nc.vector.memset(v1[:, :, 0::2, 64:128], 1.0)
            nc.vector.memset(v1[:, :, 1::2, 0:64], 1.0)
            valT = [valT_p.tile([P, N], f32r, tag=f"valT{i}", name=f"valT{i}")
                    for i in range(2)]

            # ---------- projections ----------
            def proj_qk(ft):
                with nc.named_scope(f"proj_qk{ft}"):
                    for nb in range(N // NB):
                        ps = ps_p.tile([P, NB], f32, tag=["A", "B", "V0", "V1"][nb % 4])
                        for kt in range(KT):
                            nc.tensor.matmul(
                                ps,
                                lhsT=wqk_sb[:, kt, ft * P:(ft + 1) * P],
                                rhs=xT_sb[:, kt, nb * NB:(nb + 1) * NB],
                                start=(kt == 0), stop=(kt == KT - 1),
                            )
                        nc.vector.tensor_scalar_add(
                            out=qkT[:, ft, nb * NB:(nb + 1) * NB],
                            in0=ps, scalar1=bqk_sb[:, ft:ft + 1],
                        )

            def proj_v():
                with nc.named_scope("proj_v"):
                    for mt in range(MT):
                        ps = ps_p.tile([P, NH * DV], f32, tag=["A", "B", "V0", "V1"][mt % 4])
                        for kt in range(KT):
                            nc.tensor.matmul(
                                ps,
                                lhsT=xT_sb[:, kt, mt * P:(mt + 1) * P],
                                rhs=wv_sb[:, kt, :],
                                start=(kt == 0), stop=False,
                            )
                        nc.tensor.matmul(
                            ps, lhsT=ones1, rhs=bv_sb,
                            start=False, stop=True,
                        )
                        psj = ps.rearrange("p (j d) -> p j d", d=DV)
                        nc.vector.tensor_copy(
                            out=v1[:, mt, 0::2, 0:DV], in_=psj[:, 0::2, :])
                        nc.vector.tensor_copy(
                            out=v1[:, mt, 1::2, 64:64 + DV], in_=psj[:, 1::2, :])

            # ---------- attention for one head pair over one n-chunk ----------
            def attn(p, c):
                with nc.named_scope(f"attn_p{p}_c{c}"):
                    psv = [
                        ps_p.tile([P, CH], f32, tag=f"V{oe}",
                                  name=f"psv{oe}_p{p}_c{c}")
                        for oe in (0, 1)
                    ]
                    for mt in range(MT):
                        if mt % 4 == 0:
                            em4 = mask_p.tile([P, 4, CH], bf16)
                            nc.gpsimd.dma_start(
                                out=em4,
                                in_=emT[mt * P:(mt + 4) * P,
                                        c * CH:(c + 1) * CH].rearrange(
                                    "(g p) n -> p g n", p=P),
                            )
                        em = em4[:, mt % 4, :]
                        for oe in (0, 1):
                            j = 2 * p + oe
                            base = oe * 64
                            psl = ps_p.tile([P, CH], f32, tag="AB"[oe])
                            for h2 in range(CH // NB):
                                nc.tensor.matmul(
                                    psl[:, h2 * NB:(h2 + 1) * NB],
                                    lhsT=qkT[base:base + 64, 2 + p,
                                                mt * P:(mt + 1) * P],
                                    rhs=qkT[base:base + 64, p,
                                               c * CH + h2 * NB:
                                               c * CH + (h2 + 1) * NB],
                                    start=True, stop=True,
                                )
                            wt = wT_p.tile([P, CH], bf16)
                            nc.scalar.activation(out=wt, in_=psl, func=AF.Exp)
                            nc.vector.tensor_mul(out=wt, in0=wt, in1=em)
                            for h2 in range(CH // NB):
                                nc.tensor.matmul(
                                    psv[oe][:, h2 * NB:(h2 + 1) * NB],
                                    lhsT=v1[:, mt, j, :],
                                    rhs=wt[:, h2 * NB:(h2 + 1) * NB],
                                    start=(mt == 0), stop=(mt == MT - 1),
                                )
                    # normalization: recip the replicated Z rows, DMA-move the
                    # reciprocal block to the values rows' partitions, multiply.
                    with nc.named_scope(f"norm_p{p}_c{c}"):
                        zr = z_p.tile([P, CH], f32, tag="zr")
                        zm = z_p.tile([P, CH], f32, tag="zm")
                        cs = slice(c * CH, (c + 1) * CH)
                        # even head: values rows 0:64, Z rows 64:128
                        nc.vector.reciprocal(out=zr[64:128, :], in_=psv[0][64:128, :])
                        nc.gpsimd.dma_start(out=zm[0:64, :], in_=zr[64:128, :])
                        nc.vector.tensor_mul(
                            out=valT[p][0:64, cs], in0=psv[0][0:64, :],
                            in1=zm[0:64, :],
                        )
                        # odd head: Z rows 0:64, values rows 64:128
                        nc.vector.reciprocal(out=zr[0:64, :], in_=psv[1][0:64, :])
                        nc.gpsimd.dma_start(out=zm[64:128, :], in_=zr[0:64, :])
                        nc.vector.tensor_mul(
                            out=valT[p][64:128, cs], in0=psv[1][64:128, :],
                            in1=zm[64:128, :],
                        )

            # ---------- output projection ----------
            def outproj():
                with nc.named_scope("outproj"):
                    for nt in range(MT):
                        ob = wT_p.tile([P, DSTR], f32, tag="outbuf")
                        for ds in range(DSTR // NB):
                            ps = ps_p.tile([P, NB], f32, tag="AB"[(2 * nt + ds) % 2])
                            for p in range(2):
                                nc.tensor.matmul(
                                    ps,
                                    lhsT=valT[p][:, nt * P:(nt + 1) * P],
                                    rhs=wout_sb[:, p, ds * NB:(ds + 1) * NB],
                                    start=(p == 0), stop=(p == 1),
                                )
                            obs = ob[:, ds * NB:(ds + 1) * NB]
                            if ds == 0:
                                nc.scalar.copy(out=obs, in_=ps)
                            else:
                                nc.vector.tensor_copy(out=obs, in_=ps)
                        eng = nc.sync if nt % 2 == 0 else nc.gpsimd
                        eng.dma_start(
                            out=out[nt * P:(nt + 1) * P, :], in_=ob)

            proj_qk(0)
            proj_qk(2)
            proj_v()
            for c in range(NCH):
                attn(0, c)
            proj_qk(1)
            proj_qk(3)
            for c in range(NCH):
                attn(1, c)
            outproj()

    nc.compile()
    return nc


def get_nc():
    if "nc" not in _BUILT:
        _BUILT["nc"] = _build_nc()
    return _BUILT["nc"]


def _shard_inputs(stream, mask, w_qkv, b_qkv, w_out):
    """Build the 8 per-core input maps (host-side layout transforms)."""
    stream = np.asarray(stream, np.float32)
    mask = np.asarray(mask, np.float32)
    w_qkv = np.asarray(w_qkv, np.float32)
    b_qkv = np.asarray(b_qkv, np.float32)
    w_out = np.asarray(w_out, np.float32)

    emT = np.exp(mask[0].T).astype(BF16)  # (N, N) exp of transposed mask
    xT = [np.ascontiguousarray(stream[:, b, :].T) for b in range(B)]

    in_maps = []
    for d in range(N_CORES):
        b = d // 4
        heads = [(d % 4) * 4 + j for j in range(NH)]
        qc = [w_qkv[:, h * DHEAD:h * DHEAD + DKQ] for h in heads]
        kc = [w_qkv[:, h * DHEAD + DKQ:h * DHEAD + 2 * DKQ] for h in heads]
        vc = [w_qkv[:, h * DHEAD + 2 * DKQ:(h + 1) * DHEAD] for h in heads]
        wqk = np.ascontiguousarray(np.concatenate(
            [qc[0], qc[1], qc[2], qc[3], kc[0], kc[1], kc[2], kc[3]], axis=1))
        wv = np.ascontiguousarray(np.concatenate(vc, axis=1))
        bq = [b_qkv[h * DHEAD:h * DHEAD + DKQ] for h in heads]
        bk = [b_qkv[h * DHEAD + DKQ:h * DHEAD + 2 * DKQ] for h in heads]
        bvv = [b_qkv[h * DHEAD + 2 * DKQ:(h + 1) * DHEAD] for h in heads]
        bqk_arr = np.stack(
            [np.concatenate([bq[0], bq[1]]), np.concatenate([bq[2], bq[3]]),
             np.concatenate([bk[0], bk[1]]), np.concatenate([bk[2], bk[3]])],
            axis=1).astype(np.float32)
        bv_arr = np.ascontiguousarray(np.concatenate(bvv)[None, :])
        woutd = np.ascontiguousarray(
            np.concatenate([w_out[h * DV:(h + 1) * DV, :] for h in heads], axis=0))
        in_maps.append({
            "xT": xT[b], "wqk": wqk, "wv": wv, "bqk": bqk_arr, "bv": bv_arr,
            "ones": np.ones((1, P), np.float32), "emT": emT, "wout": woutd,
        })
    return in_maps


def kernel(stream, mask, w_qkv, b_qkv, w_out, b_out):
    nc = get_nc()
    in_maps = _shard_inputs(stream, mask, w_qkv, b_qkv, w_out)
    res = run_bass_kernel_spmd(nc, in_maps, core_ids=list(range(N_CORES)))
    b_out = np.asarray(b_out, np.float32)
    out = np.empty((N, B, DSTR), np.float32)
    for b in range(B):
        acc = res.results[4 * b]["out"].copy()
        for i in range(1, 4):
            acc += res.results[4 * b + i]["out"]
        out[:, b, :] = acc + b_out
    return out


# revision 18
# speedup vs baseline: 1.0739x; 1.0739x over previous
"""MultiHeadAttention Trainium2 kernel (8-core SPMD, head/tensor parallel).

Problem (hardcoded shapes): stream (2048, 2, 1024) f32, mask (1, 2048, 2048),
w_qkv (1024, 3072), b_qkv (3072,), w_out (1024, 1024), b_out (1024,).
N=2048, B=2, HEADS=16, D_KQ=D_V=64, D_HEAD=192.

Sharding: core d handles batch b=d//4 and 4 heads [4*(d%4), 4*(d%4)+4).
Per-core compute, all in the "transposed" orientation (no on-device transposes;
the host pre-transposes stream and mask during sharding):

  qkT[f, n]   = (w_qkv_local.T @ x_b.T)[f, n] + b     (f = head-pair d dims)
  v[m, dv]    = (x_b @ w_v_local)[m, dv] + b_v
  logitsT     = k qT  per head:  lT[m, n] = sum_d k[m,d] q[n,d]
  wT[m, n]    = exp(lT) * exp(maskT)[m, n]            (softmax unnormalized)
  psv         = [v | 1]^T-style PV matmul giving values^T rows + replicated
                row-sums Z (the softmax denominator) for free
  valT[hv, n] = psv_values * (1/Z)                    (per-n normalization)
  out_partial = valT^T @ w_out_local                  (summed on host per batch)

Matmuls run as float32r (full-rate fp32 storage) except PV which is bf16
(required so the mask multiply runs at DVE 2x bf16 rate).
"""

import numpy as np
import ml_dtypes

import concourse.bass as bass
import concourse.tile as tile
from concourse import bacc, mybir
from concourse.bass_utils import run_bass_kernel_spmd

BF16 = ml_dtypes.bfloat16
dt = mybir.dt
AF = mybir.ActivationFunctionType

# Shapes (hardcoded per the problem spec)
N = 2048          # sequence length
B = 2             # batch
DSTR = 1024       # d_stream
HEADS = 16        # total heads
NH = 4            # heads per core
DKQ = 64
DV = 64
DHEAD = 2 * DKQ + DV
P = 128
KT = DSTR // P    # 8 contraction k-tiles for projections
MT = N // P       # 16 m-tiles
CH = 1024         # attention n-chunk width
NCH = N // CH     # 2 chunks
NB = 512          # matmul moving free dim
N_CORES = 8

f32, f32r, bf16 = dt.float32, dt.float32r, dt.bfloat16

_BUILT = {}


def _build_nc():
    """Build + compile the single-core SPMD Bass program (same on all cores)."""
    nc = bacc.Bacc("TRN2", target_bir_lowering=False, debug=False)

    xT = nc.dram_tensor("xT", [DSTR, N], f32r, kind="ExternalInput").ap()
    wqk = nc.dram_tensor("wqk", [DSTR, 4 * P], f32r, kind="ExternalInput").ap()
    wv = nc.dram_tensor("wv", [DSTR, NH * DV], f32r, kind="ExternalInput").ap()
    bqk = nc.dram_tensor("bqk", [P, 4], f32, kind="ExternalInput").ap()
    bv = nc.dram_tensor("bv", [1, NH * DV], f32r, kind="ExternalInput").ap()
    ones = nc.dram_tensor("ones", [1, P], f32r, kind="ExternalInput").ap()
    emT = nc.dram_tensor("emT", [N, N], bf16, kind="ExternalInput").ap()
    wout = nc.dram_tensor("wout", [NH * DV, DSTR], f32r, kind="ExternalInput").ap()
    out = nc.dram_tensor("out", [N, DSTR], f32, kind="ExternalOutput").ap()

    with tile.TileContext(nc) as tc:
        with (
            tc.tile_pool(name="consts", bufs=1) as consts,
            tc.tile_pool(name="xw", bufs=1) as xw_p,
            tc.tile_pool(name="qkT", bufs=1) as qkT_p,
            tc.tile_pool(name="v1", bufs=1) as v1_p,
            tc.tile_pool(name="valT", bufs=1) as valT_p,
            tc.tile_pool(name="mask", bufs=2) as mask_p,
            tc.tile_pool(name="wT", bufs=2) as wT_p,
            tc.tile_pool(name="z", bufs=1) as z_p,
            tc.tile_pool(name="ps", bufs=1, space="PSUM") as ps_p,
        ):
            # ---------- persistent SBUF ----------
            bqk_sb = consts.tile([P, 4], f32)
            nc.sync.dma_start(out=bqk_sb, in_=bqk)
            bv_sb = consts.tile([1, NH * DV], f32r)
            nc.sync.dma_start(out=bv_sb, in_=bv)
            ones1 = consts.tile([1, P], f32r)
            nc.sync.dma_start(out=ones1, in_=ones)
            wout_sb = consts.tile([P, 2, DSTR], f32r)
            nc.sync.dma_start(
                out=wout_sb, in_=wout.rearrange("(i p) d -> p i d", p=P))

            xT_sb = xw_p.tile([P, KT, N], f32r)
            wqk_sb = xw_p.tile([P, KT, 4 * P], f32r)
            wv_sb = xw_p.tile([P, KT, NH * DV], f32r)
            nc.sync.dma_start(
                out=wqk_sb, in_=wqk.rearrange("(kt p) f -> p kt f", p=P))
            nc.sync.dma_start(
                out=wv_sb, in_=wv.rearrange("(kt p) f -> p kt f", p=P))
            for g in range(KT // 2):
                nc.sync.dma_start(
                    out=xT_sb[:, 2 * g:2 * g + 2, :],
                    in_=xT[g * 2 * P:(g + 1) * 2 * P, :].rearrange(
                        "(kt p) n -> p kt n", p=P))

            # qkT f-tiles: 0 = q pair0, 1 = q pair1, 2 = k pair0, 3 = k pair1
            # (within a tile: partitions 0:64 = even head's d, 64:128 = odd head's)
            qkT = qkT_p.tile([P, 4, N], f32r)
            # v1 lhsT slots per (mt, head): even-in-pair = [v | ones],
            # odd-in-pair = [ones | v] -> PV output carries values rows and
            # 64x-replicated Z rows in complementary partition halves.
            v1 = v1_p.tile([P, MT, NH, P], bf16)
            nc.vector.memset(v1, 0.0)
            nc.vector.memset(v1[:, :, 0::2, 64:128], 1.0)
            nc.vector.memset(v1[:, :, 1::2, 0:64], 1.0)
            valT = [valT_p.tile([P, N], f32r, tag=f"valT{i}", name=f"valT{i}")
                    for i in range(2)]

            # ---------- projections ----------
            def proj_qk(ft):
                with nc.named_scope(f"proj_qk{ft}"):
                    for nb in range(N // NB):
                        ps = ps_p.tile([P, NB], f32, tag=["A", "B", "V0", "V1"][nb % 4])
                        for kt in range(KT):
                            nc.tensor.matmul(
                                ps,
                                lhsT=wqk_sb[:, kt, ft * P:(ft + 1) * P],
                                rhs=xT_sb[:, kt, nb * NB:(nb + 1) * NB],
                                start=(kt == 0), stop=(kt == KT - 1),
                            )
                        nc.vector.tensor_scalar_add(
                            out=qkT[:, ft, nb * NB:(nb + 1) * NB],
                            in0=ps, scalar1=bqk_sb[:, ft:ft + 1],
                        )

            def proj_v():
                with nc.named_scope("proj_v"):
                    for mt in range(MT):
                        ps = ps_p.tile([P, NH * DV], f32, tag=["A", "B", "V0", "V1"][mt % 4])
                        for kt in range(KT):
                            nc.tensor.matmul(
                                ps,
                                lhsT=xT_sb[:, kt, mt * P:(mt + 1) * P],
                                rhs=wv_sb[:, kt, :],
                                start=(kt == 0), stop=False,
                            )
                        nc.tensor.matmul(
                            ps, lhsT=ones1, rhs=bv_sb,
                            start=False, stop=True,
                        )
                        psj = ps.rearrange("p (j d) -> p j d", d=DV)
                        nc.vector.tensor_copy(
                            out=v1[:, mt, 0::2, 0:DV], in_=psj[:, 0::2, :])
                        nc.vector.tensor_copy(
                            out=v1[:, mt, 1::2, 64:64 + DV], in_=psj[:, 1::2, :])

            # ---------- attention for one head pair over one n-chunk ----------
            def attn(p, c):
                with nc.named_scope(f"attn_p{p}_c{c}"):
                    psv = [
                        ps_p.tile([P, CH], f32, tag=f"V{oe}",
                                  name=f"psv{oe}_p{p}_c{c}")
                        for oe in (0, 1)
                    ]
                    for mt in range(MT):
                        if mt % 4 == 0:
                            em4 = mask_p.tile([P, 4, CH], bf16)
                            nc.sync.dma_start(
                                out=em4,
                                in_=emT[mt * P:(mt + 4) * P,
                                        c * CH:(c + 1) * CH].rearrange(
                                    "(g p) n -> p g n", p=P),
                            )
                        em = em4[:, mt % 4, :]
                        for oe in (0, 1):
                            j = 2 * p + oe
                            base = oe * 64
                            psl = ps_p.tile([P, CH], f32, tag="AB"[oe])
                            for h2 in range(CH // NB):
                                nc.tensor.matmul(
                                    psl[:, h2 * NB:(h2 + 1) * NB],
                                    lhsT=qkT[base:base + 64, 2 + p,
                                                mt * P:(mt + 1) * P],
                                    rhs=qkT[base:base + 64, p,
                                               c * CH + h2 * NB:
                                               c * CH + (h2 + 1) * NB],
                                    start=True, stop=True,
                                )
                            wt = wT_p.tile([P, CH], bf16)
                            nc.scalar.activation(out=wt, in_=psl, func=AF.Exp)
                            nc.vector.tensor_mul(out=wt, in0=wt, in1=em)
                            for h2 in range(CH // NB):
                                nc.tensor.matmul(
                                    psv[oe][:, h2 * NB:(h2 + 1) * NB],
                                    lhsT=v1[:, mt, j, :],
                                    rhs=wt[:, h2 * NB:(h2 + 1) * NB],
                                    start=(mt == 0), stop=(mt == MT - 1),
                                )
                    # normalization: recip the replicated Z rows, DMA-move the
                    # reciprocal block to the values rows' partitions, multiply.
                    with nc.named_scope(f"norm_p{p}_c{c}"):
                        zr = z_p.tile([P, CH], f32, tag="zr")
                        zm = z_p.tile([P, CH], f32, tag="zm")
                        cs = slice(c * CH, (c + 1) * CH)
                        # even head: values rows 0:64, Z rows 64:128
                        nc.vector.reciprocal(out=zr[64:128, :], in_=psv[0][64:128, :])
                        nc.sync.dma_start(out=zm[0:64, :], in_=zr[64:128, :])
                        nc.vector.tensor_mul(
                            out=valT[p][0:64, cs], in0=psv[0][0:64, :],
                            in1=zm[0:64, :],
                        )
                        # odd head: Z rows 0:64, values rows 64:128
                        nc.vector.reciprocal(out=zr[0:64, :], in_=psv[1][0:64, :])
                        nc.sync.dma_start(out=zm[64:128, :], in_=zr[0:64, :])
                        nc.vector.tensor_mul(
                            out=valT[p][64:128, cs], in0=psv[1][64:128, :],
                            in1=zm[64:128, :],
                        )

            # ---------- output projection ----------
            def outproj():
                with nc.named_scope("outproj"):
                    for nt2 in range(MT // 2):
                        ob = wT_p.tile([P, 2, DSTR], f32, tag="outbuf")
                        for half in range(2):
                            nt = 2 * nt2 + half
                            for ds in range(DSTR // NB):
                                ps = ps_p.tile([P, NB], f32,
                                               tag="AB"[(2 * nt + ds) % 2])
                                for p in range(2):
                                    nc.tensor.matmul(
                                        ps,
                                        lhsT=valT[p][:, nt * P:(nt + 1) * P],
                                        rhs=wout_sb[:, p, ds * NB:(ds + 1) * NB],
                                        start=(p == 0), stop=(p == 1),
                                    )
                                obs = ob[:, half, ds * NB:(ds + 1) * NB]
                                if ds == 0:
                                    nc.scalar.copy(out=obs, in_=ps)
                                else:
                                    nc.vector.tensor_copy(out=obs, in_=ps)
                        eng = nc.sync if nt2 % 2 == 0 else nc.scalar
                        eng.dma_start(
                            out=out[nt2 * 2 * P:(nt2 + 1) * 2 * P, :].rearrange(
                                "(h p) d -> p h d", p=P),
                            in_=ob)

            proj_qk(0)
            proj_qk(2)
            proj_v()
            for c in range(NCH):
                attn(0, c)
            proj_qk(1)
            proj_qk(3)
            for c in range(NCH):
                attn(1, c)
            outproj()

    nc.compile()
    return nc


def get_nc():
    if "nc" not in _BUILT:
        _BUILT["nc"] = _build_nc()
    return _BUILT["nc"]


def _shard_inputs(stream, mask, w_qkv, b_qkv, w_out):
    """Build the 8 per-core input maps (host-side layout transforms)."""
    stream = np.asarray(stream, np.float32)
    mask = np.asarray(mask, np.float32)
    w_qkv = np.asarray(w_qkv, np.float32)
    b_qkv = np.asarray(b_qkv, np.float32)
    w_out = np.asarray(w_out, np.float32)

    emT = np.exp(mask[0].T).astype(BF16)  # (N, N) exp of transposed mask
    xT = [np.ascontiguousarray(stream[:, b, :].T) for b in range(B)]

    in_maps = []
    for d in range(N_CORES):
        b = d // 4
        heads = [(d % 4) * 4 + j for j in range(NH)]
        qc = [w_qkv[:, h * DHEAD:h * DHEAD + DKQ] for h in heads]
        kc = [w_qkv[:, h * DHEAD + DKQ:h * DHEAD + 2 * DKQ] for h in heads]
        vc = [w_qkv[:, h * DHEAD + 2 * DKQ:(h + 1) * DHEAD] for h in heads]
        wqk = np.ascontiguousarray(np.concatenate(
            [qc[0], qc[1], qc[2], qc[3], kc[0], kc[1], kc[2], kc[3]], axis=1))
        wv = np.ascontiguousarray(np.concatenate(vc, axis=1))
        bq = [b_qkv[h * DHEAD:h * DHEAD + DKQ] for h in heads]
        bk = [b_qkv[h * DHEAD + DKQ:h * DHEAD + 2 * DKQ] for h in heads]
        bvv = [b_qkv[h * DHEAD + 2 * DKQ:(h + 1) * DHEAD] for h in heads]
        bqk_arr = np.stack(
            [np.concatenate([bq[0], bq[1]]), np.concatenate([bq[2], bq[3]]),
             np.concatenate([bk[0], bk[1]]), np.concatenate([bk[2], bk[3]])],
            axis=1).astype(np.float32)
        bv_arr = np.ascontiguousarray(np.concatenate(bvv)[None, :])
        woutd = np.ascontiguousarray(
            np.concatenate([w_out[h * DV:(h + 1) * DV, :] for h in heads], axis=0))
        in_maps.append({
            "xT": xT[b], "wqk": wqk, "wv": wv, "bqk": bqk_arr, "bv": bv_arr,
            "ones": np.ones((1, P), np.float32), "emT": emT, "wout": woutd,
        })
    return in_maps


def kernel(stream, mask, w_qkv, b_qkv, w_out, b_out):
    nc = get_nc()
    in_maps = _shard_inputs(stream, mask, w_qkv, b_qkv, w_out)
    res = run_bass_kernel_spmd(nc, in_maps, core_ids=list(range(N_CORES)))
    b_out = np.asarray(b_out, np.float32)
    out = np.empty((N, B, DSTR), np.float32)
    for b in range(B):
        acc = res.results[4 * b]["out"].copy()
        for i in range(1, 4):
            acc += res.results[4 * b + i]["out"]
        out[:, b, :] = acc + b_out
    return out


# revision 27
# speedup vs baseline: 1.2535x; 1.1672x over previous
"""MultiHeadAttention Trainium2 kernel (8-core SPMD, head/tensor parallel).

Problem (hardcoded shapes): stream (2048, 2, 1024) f32, mask (1, 2048, 2048),
w_qkv (1024, 3072), b_qkv (3072,), w_out (1024, 1024), b_out (1024,).
N=2048, B=2, HEADS=16, D_KQ=D_V=64, D_HEAD=192.

Sharding: core d handles batch b=d//4 and 4 heads [4*(d%4), 4*(d%4)+4).
Per-core compute, all in the "transposed" orientation (no on-device transposes;
the host pre-transposes stream and mask during sharding):

  qkT[f, n]   = (w_qkv_local.T @ x_b.T)[f, n] + b     (f = head-pair d dims)
  v[m, dv]    = (x_b @ w_v_local)[m, dv] + b_v
  logitsT     = k qT  per head:  lT[m, n] = sum_d k[m,d] q[n,d]
  wT[m, n]    = exp(lT) * exp(maskT)[m, n]            (softmax unnormalized)
  psv         = [v | 1]^T-style PV matmul giving values^T rows + replicated
                row-sums Z (the softmax denominator) for free
  valT[hv, n] = psv_values * (1/Z)                    (per-n normalization)
  out_partial = valT^T @ w_out_local                  (summed on host per batch)

Matmuls run as float32r (full-rate fp32 storage) except PV which is bf16
(required so the mask multiply runs at DVE 2x bf16 rate).
"""

import numpy as np
import ml_dtypes

import concourse.bass as bass
import concourse.tile as tile
from concourse import bacc, mybir
from concourse.bass_utils import run_bass_kernel_spmd

BF16 = ml_dtypes.bfloat16
dt = mybir.dt
AF = mybir.ActivationFunctionType

# Shapes (hardcoded per the problem spec)
N = 2048          # sequence length
B = 2             # batch
DSTR = 1024       # d_stream
HEADS = 16        # total heads
NH = 4            # heads per core
DKQ = 64
DV = 64
DHEAD = 2 * DKQ + DV
P = 128
KT = DSTR // P    # 8 contraction k-tiles for projections
MT = N // P       # 16 m-tiles
CH = 1024         # attention n-chunk width
NCH = N // CH     # 2 chunks
NB = 512          # matmul moving free dim
N_CORES = 8

f32, f32r, bf16 = dt.float32, dt.float32r, dt.bfloat16

_BUILT = {}


def _build_nc():
    """Build + compile the single-core SPMD Bass program (same on all cores)."""
    nc = bacc.Bacc("TRN2", target_bir_lowering=False, debug=False)

    xT = nc.dram_tensor("xT", [DSTR, N], f32r, kind="ExternalInput").ap()
    wqk = nc.dram_tensor("wqk", [DSTR, 4 * P], f32r, kind="ExternalInput").ap()
    wv = nc.dram_tensor("wv", [DSTR, NH * DV], f32r, kind="ExternalInput").ap()
    bqk = nc.dram_tensor("bqk", [P, 4], f32, kind="ExternalInput").ap()
    bv = nc.dram_tensor("bv", [1, NH * DV], f32r, kind="ExternalInput").ap()
    ones = nc.dram_tensor("ones", [1, P], f32r, kind="ExternalInput").ap()
    emT = nc.dram_tensor("emT", [N, N], bf16, kind="ExternalInput").ap()
    wout = nc.dram_tensor("wout", [NH * DV, DSTR], f32r, kind="ExternalInput").ap()
    out = nc.dram_tensor("out", [N, DSTR], f32, kind="ExternalOutput").ap()

    with tile.TileContext(nc) as tc:
        with (
            tc.tile_pool(name="consts", bufs=1) as consts,
            tc.tile_pool(name="xw", bufs=1) as xw_p,
            tc.tile_pool(name="qkT", bufs=1) as qkT_p,
            tc.tile_pool(name="v1", bufs=1) as v1_p,
            tc.tile_pool(name="valT", bufs=1) as valT_p,
            tc.tile_pool(name="mask", bufs=3) as mask_p,
            tc.tile_pool(name="wT", bufs=3) as wT_p,
            tc.tile_pool(name="z", bufs=1) as z_p,
            tc.tile_pool(name="ps", bufs=1, space="PSUM") as ps_p,
        ):
            # ---------- persistent SBUF ----------
            bqk_sb = consts.tile([P, 4], f32)
            nc.sync.dma_start(out=bqk_sb, in_=bqk)
            bv_sb = consts.tile([1, NH * DV], f32r)
            nc.sync.dma_start(out=bv_sb, in_=bv)
            ones1 = consts.tile([1, P], f32r)
            nc.sync.dma_start(out=ones1, in_=ones)
            wout_sb = consts.tile([P, 2, DSTR], f32r)
            nc.sync.dma_start(
                out=wout_sb, in_=wout.rearrange("(i p) d -> p i d", p=P))

            xT_sb = xw_p.tile([P, KT, N], f32r)
            wqk_sb = xw_p.tile([P, KT, 4 * P], f32r)
            wv_sb = xw_p.tile([P, KT, NH * DV], f32r)
            for kt in range(KT):
                e1 = nc.sync if kt % 2 == 0 else nc.scalar
                e2 = nc.scalar if kt % 2 == 0 else nc.sync
                e1.dma_start(out=xT_sb[:, kt, :], in_=xT[kt * P:(kt + 1) * P, :])
                e2.dma_start(out=wqk_sb[:, kt, :], in_=wqk[kt * P:(kt + 1) * P, :])
                e2.dma_start(out=wv_sb[:, kt, :], in_=wv[kt * P:(kt + 1) * P, :])

            # qkT f-tiles: 0 = q pair0, 1 = q pair1, 2 = k pair0, 3 = k pair1
            # (within a tile: partitions 0:64 = even head's d, 64:128 = odd head's)
            qkT = qkT_p.tile([P, 4, N], f32r)
            # v1 lhsT slots per (mt, head): even-in-pair = [v | ones],
            # odd-in-pair = [ones | v] -> PV output carries values rows and
            # 64x-replicated Z rows in complementary partition halves.
            v1 = v1_p.tile([P, MT, NH, P], bf16)
            nc.vector.memset(v1, 0.0)
            nc.vector.memset(v1[:, :, 0::2, 64:128], 1.0)
            nc.vector.memset(v1[:, :, 1::2, 0:64], 1.0)
            valT = [valT_p.tile([P, N], f32r, tag=f"valT{i}", name=f"valT{i}")
                    for i in range(2)]

            # ---------- projections ----------
            def proj_qk(ft):
                with nc.named_scope(f"proj_qk{ft}"):
                    for nb in range(N // NB):
                        ps = ps_p.tile([P, NB], f32, tag=["A", "B", "V0", "V1"][nb % 4])
                        for kt in range(KT):
                            nc.tensor.matmul(
                                ps,
                                lhsT=wqk_sb[:, kt, ft * P:(ft + 1) * P],
                                rhs=xT_sb[:, kt, nb * NB:(nb + 1) * NB],
                                start=(kt == 0), stop=(kt == KT - 1),
                            )
                        nc.vector.tensor_scalar_add(
                            out=qkT[:, ft, nb * NB:(nb + 1) * NB],
                            in0=ps, scalar1=bqk_sb[:, ft:ft + 1],
                        )

            def proj_v():
                with nc.named_scope("proj_v"):
                    for mt in range(MT):
                        ps = ps_p.tile([P, NH * DV], f32, tag=["A", "B", "V0", "V1"][mt % 4])
                        for kt in range(KT):
                            nc.tensor.matmul(
                                ps,
                                lhsT=xT_sb[:, kt, mt * P:(mt + 1) * P],
                                rhs=wv_sb[:, kt, :],
                                start=(kt == 0), stop=False,
                            )
                        nc.tensor.matmul(
                            ps, lhsT=ones1, rhs=bv_sb,
                            start=False, stop=True,
                        )
                        psj = ps.rearrange("p (j d) -> p j d", d=DV)
                        nc.vector.tensor_copy(
                            out=v1[:, mt, 0::2, 0:DV], in_=psj[:, 0::2, :])
                        nc.vector.tensor_copy(
                            out=v1[:, mt, 1::2, 64:64 + DV], in_=psj[:, 1::2, :])

            # ---------- attention for one head pair over one n-chunk ----------
            def attn(p, c):
                with nc.named_scope(f"attn_p{p}_c{c}"):
                    psv = [
                        ps_p.tile([P, CH], f32, tag=f"V{oe}",
                                  name=f"psv{oe}_p{p}_c{c}")
                        for oe in (0, 1)
                    ]
                    for mt in range(MT):
                        em = mask_p.tile([P, CH], bf16)
                        nc.sync.dma_start(
                            out=em,
                            in_=emT[mt * P:(mt + 1) * P, c * CH:(c + 1) * CH],
                        )
                        for oe in (0, 1):
                            j = 2 * p + oe
                            base = oe * 64
                            psl = ps_p.tile([P, CH], f32, tag="AB"[oe])
                            for h2 in range(CH // NB):
                                nc.tensor.matmul(
                                    psl[:, h2 * NB:(h2 + 1) * NB],
                                    lhsT=qkT[base:base + 64, 2 + p,
                                                mt * P:(mt + 1) * P],
                                    rhs=qkT[base:base + 64, p,
                                               c * CH + h2 * NB:
                                               c * CH + (h2 + 1) * NB],
                                    start=True, stop=True,
                                )
                            wt = wT_p.tile([P, CH], bf16)
                            nc.scalar.activation(out=wt, in_=psl, func=AF.Exp)
                            nc.vector.tensor_mul(out=wt, in0=wt, in1=em)
                            for h2 in range(CH // NB):
                                nc.tensor.matmul(
                                    psv[oe][:, h2 * NB:(h2 + 1) * NB],
                                    lhsT=v1[:, mt, j, :],
                                    rhs=wt[:, h2 * NB:(h2 + 1) * NB],
                                    start=(mt == 0), stop=(mt == MT - 1),
                                )
                    # normalization: recip the replicated Z rows, DMA-move the
                    # reciprocal block to the values rows' partitions, multiply.
                    with nc.named_scope(f"norm_p{p}_c{c}"):
                        zr = z_p.tile([P, CH], f32, tag="zr")
                        zm = z_p.tile([P, CH], f32, tag="zm")
                        cs = slice(c * CH, (c + 1) * CH)
                        # even head: values rows 0:64, Z rows 64:128
                        nc.vector.reciprocal(out=zr[64:128, :], in_=psv[0][64:128, :])
                        nc.sync.dma_start(out=zm[0:64, :], in_=zr[64:128, :])
                        nc.vector.tensor_mul(
                            out=valT[p][0:64, cs], in0=psv[0][0:64, :],
                            in1=zm[0:64, :],
                        )
                        # odd head: Z rows 0:64, values rows 64:128
                        nc.vector.reciprocal(out=zr[0:64, :], in_=psv[1][0:64, :])
                        nc.sync.dma_start(out=zm[64:128, :], in_=zr[0:64, :])
                        nc.vector.tensor_mul(
                            out=valT[p][64:128, cs], in0=psv[1][64:128, :],
                            in1=zm[64:128, :],
                        )

            # ---------- output projection ----------
            def outproj():
                with nc.named_scope("outproj"):
                    for nt2 in range(MT // 2):
                        ob = wT_p.tile([P, 2, DSTR], f32, tag="outbuf")
                        for half in range(2):
                            nt = 2 * nt2 + half
                            for ds in range(DSTR // NB):
                                ps = ps_p.tile([P, NB], f32,
                                               tag="AB"[(2 * nt + ds) % 2])
                                for p in range(2):
                                    nc.tensor.matmul(
                                        ps,
                                        lhsT=valT[p][:, nt * P:(nt + 1) * P],
                                        rhs=wout_sb[:, p, ds * NB:(ds + 1) * NB],
                                        start=(p == 0), stop=(p == 1),
                                    )
                                obs = ob[:, half, ds * NB:(ds + 1) * NB]
                                if ds == 0:
                                    nc.scalar.copy(out=obs, in_=ps)
                                else:
                                    nc.vector.tensor_copy(out=obs, in_=ps)
                        eng = nc.sync if nt2 % 2 == 0 else nc.scalar
                        eng.dma_start(
                            out=out[nt2 * 2 * P:(nt2 + 1) * 2 * P, :].rearrange(
                                "(h p) d -> p h d", p=P),
                            in_=ob)

            proj_qk(0)
            proj_qk(2)
            proj_v()
            for c in range(NCH):
                attn(0, c)
            proj_qk(1)
            proj_qk(3)
            for c in range(NCH):
                attn(1, c)
            outproj()

    nc.compile()
    return nc


def get_nc():
    if "nc" not in _BUILT:
        _BUILT["nc"] = _build_nc()
    return _BUILT["nc"]


def _shard_inputs(stream, mask, w_qkv, b_qkv, w_out):
    """Build the 8 per-core input maps (host-side layout transforms)."""
    stream = np.asarray(stream, np.float32)
    mask = np.asarray(mask, np.float32)
    w_qkv = np.asarray(w_qkv, np.float32)
    b_qkv = np.asarray(b_qkv, np.float32)
    w_out = np.asarray(w_out, np.float32)

    emT = np.exp(mask[0].T).astype(BF16)  # (N, N) exp of transposed mask
    xT = [np.ascontiguousarray(stream[:, b, :].T) for b in range(B)]

    in_maps = []
    for d in range(N_CORES):
        b = d // 4
        heads = [(d % 4) * 4 + j for j in range(NH)]
        qc = [w_qkv[:, h * DHEAD:h * DHEAD + DKQ] for h in heads]
        kc = [w_qkv[:, h * DHEAD + DKQ:h * DHEAD + 2 * DKQ] for h in heads]
        vc = [w_qkv[:, h * DHEAD + 2 * DKQ:(h + 1) * DHEAD] for h in heads]
        wqk = np.ascontiguousarray(np.concatenate(
            [qc[0], qc[1], qc[2], qc[3], kc[0], kc[1], kc[2], kc[3]], axis=1))
        wv = np.ascontiguousarray(np.concatenate(vc, axis=1))
        bq = [b_qkv[h * DHEAD:h * DHEAD + DKQ] for h in heads]
        bk = [b_qkv[h * DHEAD + DKQ:h * DHEAD + 2 * DKQ] for h in heads]
        bvv = [b_qkv[h * DHEAD + 2 * DKQ:(h + 1) * DHEAD] for h in heads]
        bqk_arr = np.stack(
            [np.concatenate([bq[0], bq[1]]), np.concatenate([bq[2], bq[3]]),
             np.concatenate([bk[0], bk[1]]), np.concatenate([bk[2], bk[3]])],
            axis=1).astype(np.float32)
        bv_arr = np.ascontiguousarray(np.concatenate(bvv)[None, :])
        woutd = np.ascontiguousarray(
            np.concatenate([w_out[h * DV:(h + 1) * DV, :] for h in heads], axis=0))
        in_maps.append({
            "xT": xT[b], "wqk": wqk, "wv": wv, "bqk": bqk_arr, "bv": bv_arr,
            "ones": np.ones((1, P), np.float32), "emT": emT, "wout": woutd,
        })
    return in_maps


def kernel(stream, mask, w_qkv, b_qkv, w_out, b_out):
    nc = get_nc()
    in_maps = _shard_inputs(stream, mask, w_qkv, b_qkv, w_out)
    res = run_bass_kernel_spmd(nc, in_maps, core_ids=list(range(N_CORES)))
    b_out = np.asarray(b_out, np.float32)
    out = np.empty((N, B, DSTR), np.float32)
    for b in range(B):
        acc = res.results[4 * b]["out"].copy()
        for i in range(1, 4):
            acc += res.results[4 * b + i]["out"]
        out[:, b, :] = acc + b_out
    return out


# revision 30
# speedup vs baseline: 1.2550x; 1.0012x over previous
"""MultiHeadAttention Trainium2 kernel (8-core SPMD, head/tensor parallel).

Problem (hardcoded shapes): stream (2048, 2, 1024) f32, mask (1, 2048, 2048),
w_qkv (1024, 3072), b_qkv (3072,), w_out (1024, 1024), b_out (1024,).
N=2048, B=2, HEADS=16, D_KQ=D_V=64, D_HEAD=192.

Sharding: core d handles batch b=d//4 and 4 heads [4*(d%4), 4*(d%4)+4).
Per-core compute, all in the "transposed" orientation (no on-device transposes;
the host pre-transposes stream and mask during sharding):

  qkT[f, n]   = (w_qkv_local.T @ x_b.T)[f, n] + b     (f = head-pair d dims)
  v[m, dv]    = (x_b @ w_v_local)[m, dv] + b_v
  logitsT     = k qT  per head:  lT[m, n] = sum_d k[m,d] q[n,d]
  wT[m, n]    = exp(lT) * exp(maskT)[m, n]            (softmax unnormalized)
  psv         = [v | 1]^T-style PV matmul giving values^T rows + replicated
                row-sums Z (the softmax denominator) for free
  valT[hv, n] = psv_values * (1/Z)                    (per-n normalization)
  out_partial = valT^T @ w_out_local                  (summed on host per batch)

Matmuls run as float32r (full-rate fp32 storage) except PV which is bf16
(required so the mask multiply runs at DVE 2x bf16 rate).
"""

import numpy as np
import ml_dtypes

import concourse.bass as bass
import concourse.tile as tile
from concourse import bacc, mybir
from concourse.bass_utils import run_bass_kernel_spmd

BF16 = ml_dtypes.bfloat16
dt = mybir.dt
AF = mybir.ActivationFunctionType

# Shapes (hardcoded per the problem spec)
N = 2048          # sequence length
B = 2             # batch
DSTR = 1024       # d_stream
HEADS = 16        # total heads
NH = 4            # heads per core
DKQ = 64
DV = 64
DHEAD = 2 * DKQ + DV
P = 128
KT = DSTR // P    # 8 contraction k-tiles for projections
MT = N // P       # 16 m-tiles
CH = 1024         # attention n-chunk width
NCH = N // CH     # 2 chunks
NB = 512          # matmul moving free dim
N_CORES = 8

f32, f32r, bf16 = dt.float32, dt.float32r, dt.bfloat16

_BUILT = {}


def _build_nc():
    """Build + compile the single-core SPMD Bass program (same on all cores)."""
    nc = bacc.Bacc("TRN2", target_bir_lowering=False, debug=False)

    xT = nc.dram_tensor("xT", [DSTR, N], f32r, kind="ExternalInput").ap()
    wqk = nc.dram_tensor("wqk", [DSTR, 4 * P], f32r, kind="ExternalInput").ap()
    wv = nc.dram_tensor("wv", [DSTR, NH * DV], f32r, kind="ExternalInput").ap()
    bqk = nc.dram_tensor("bqk", [P, 4], f32, kind="ExternalInput").ap()
    bv = nc.dram_tensor("bv", [1, NH * DV], f32r, kind="ExternalInput").ap()
    ones = nc.dram_tensor("ones", [1, P], f32r, kind="ExternalInput").ap()
    emT = nc.dram_tensor("emT", [N, N], bf16, kind="ExternalInput").ap()
    wout = nc.dram_tensor("wout", [NH * DV, DSTR], f32r, kind="ExternalInput").ap()
    out = nc.dram_tensor("out", [N, DSTR], f32, kind="ExternalOutput").ap()

    with tile.TileContext(nc) as tc:
        with (
            tc.tile_pool(name="consts", bufs=1) as consts,
            tc.tile_pool(name="xw", bufs=1) as xw_p,
            tc.tile_pool(name="qkT", bufs=1) as qkT_p,
            tc.tile_pool(name="v1", bufs=1) as v1_p,
            tc.tile_pool(name="valT", bufs=1) as valT_p,
            tc.tile_pool(name="mask", bufs=3) as mask_p,
            tc.tile_pool(name="wT", bufs=3) as wT_p,
            tc.tile_pool(name="z", bufs=1) as z_p,
            tc.tile_pool(name="ps", bufs=1, space="PSUM") as ps_p,
        ):
            # ---------- persistent SBUF ----------
            bqk_sb = consts.tile([P, 4], f32)
            nc.sync.dma_start(out=bqk_sb, in_=bqk)
            bv_sb = consts.tile([1, NH * DV], f32r)
            nc.sync.dma_start(out=bv_sb, in_=bv)
            ones1 = consts.tile([1, P], f32r)
            nc.sync.dma_start(out=ones1, in_=ones)
            wout_sb = consts.tile([P, 2, DSTR], f32r)
            nc.sync.dma_start(
                out=wout_sb, in_=wout.rearrange("(i p) d -> p i d", p=P))

            xT_sb = xw_p.tile([P, KT, N], f32r)
            wqk_sb = xw_p.tile([P, KT, 4 * P], f32r)
            wv_sb = xw_p.tile([P, KT, NH * DV], f32r)
            for kt in range(KT):
                e1 = nc.sync if kt % 2 == 0 else nc.scalar
                e2 = nc.scalar if kt % 2 == 0 else nc.sync
                e1.dma_start(out=xT_sb[:, kt, :], in_=xT[kt * P:(kt + 1) * P, :])
                e2.dma_start(out=wqk_sb[:, kt, :], in_=wqk[kt * P:(kt + 1) * P, :])
                e2.dma_start(out=wv_sb[:, kt, :], in_=wv[kt * P:(kt + 1) * P, :])

            # qkT f-tiles: 0 = q pair0, 1 = q pair1, 2 = k pair0, 3 = k pair1
            # (within a tile: partitions 0:64 = even head's d, 64:128 = odd head's)
            qkT = qkT_p.tile([P, 4, N], f32r)
            # v1 lhsT slots per (mt, head): even-in-pair = [v | ones],
            # odd-in-pair = [ones | v] -> PV output carries values rows and
            # 64x-replicated Z rows in complementary partition halves.
            v1 = v1_p.tile([P, MT, NH, P], bf16)
            nc.vector.memset(v1, 0.0)
            nc.vector.memset(v1[:, :, 0::2, 64:128], 1.0)
            nc.vector.memset(v1[:, :, 1::2, 0:64], 1.0)
            valT = [valT_p.tile([P, N], f32r, tag=f"valT{i}", name=f"valT{i}")
                    for i in range(2)]

            # ---------- projections ----------
            def proj_qk(ft):
                with nc.named_scope(f"proj_qk{ft}"):
                    for nb in range(N // NB):
                        ps = ps_p.tile([P, NB], f32, tag=["A", "B", "V0", "V1"][nb % 4])
                        for kt in range(KT):
                            nc.tensor.matmul(
                                ps,
                                lhsT=wqk_sb[:, kt, ft * P:(ft + 1) * P],
                                rhs=xT_sb[:, kt, nb * NB:(nb + 1) * NB],
                                start=(kt == 0), stop=(kt == KT - 1),
                            )
                        nc.scalar.activation(
                            out=qkT[:, ft, nb * NB:(nb + 1) * NB], in_=ps,
                            func=AF.Identity, bias=bqk_sb[:, ft:ft + 1],
                        )

            def proj_v():
                with nc.named_scope("proj_v"):
                    for mt in range(MT):
                        ps = ps_p.tile([P, NH * DV], f32, tag=["A", "B", "V0", "V1"][mt % 4])
                        for kt in range(KT):
                            nc.tensor.matmul(
                                ps,
                                lhsT=xT_sb[:, kt, mt * P:(mt + 1) * P],
                                rhs=wv_sb[:, kt, :],
                                start=(kt == 0), stop=False,
                            )
                        nc.tensor.matmul(
                            ps, lhsT=ones1, rhs=bv_sb,
                            start=False, stop=True,
                        )
                        psj = ps.rearrange("p (j d) -> p j d", d=DV)
                        nc.vector.tensor_copy(
                            out=v1[:, mt, 0::2, 0:DV], in_=psj[:, 0::2, :])
                        nc.vector.tensor_copy(
                            out=v1[:, mt, 1::2, 64:64 + DV], in_=psj[:, 1::2, :])

            # ---------- attention for one head pair over one n-chunk ----------
            def attn(p, c):
                with nc.named_scope(f"attn_p{p}_c{c}"):
                    psv = [
                        ps_p.tile([P, CH], f32, tag=f"V{oe}",
                                  name=f"psv{oe}_p{p}_c{c}")
                        for oe in (0, 1)
                    ]
                    for mt in range(MT):
                        em = mask_p.tile([P, CH], bf16)
                        nc.sync.dma_start(
                            out=em,
                            in_=emT[mt * P:(mt + 1) * P, c * CH:(c + 1) * CH],
                        )
                        for oe in (0, 1):
                            j = 2 * p + oe
                            base = oe * 64
                            psl = ps_p.tile([P, CH], f32, tag="AB"[oe])
                            for h2 in range(CH // NB):
                                nc.tensor.matmul(
                                    psl[:, h2 * NB:(h2 + 1) * NB],
                                    lhsT=qkT[base:base + 64, 2 + p,
                                                mt * P:(mt + 1) * P],
                                    rhs=qkT[base:base + 64, p,
                                               c * CH + h2 * NB:
                                               c * CH + (h2 + 1) * NB],
                                    start=True, stop=True,
                                )
                            wt = wT_p.tile([P, CH], bf16)
                            nc.scalar.activation(out=wt, in_=psl, func=AF.Exp)
                            nc.vector.tensor_mul(out=wt, in0=wt, in1=em)
                            for h2 in range(CH // NB):
                                nc.tensor.matmul(
                                    psv[oe][:, h2 * NB:(h2 + 1) * NB],
                                    lhsT=v1[:, mt, j, :],
                                    rhs=wt[:, h2 * NB:(h2 + 1) * NB],
                                    start=(mt == 0), stop=(mt == MT - 1),
                                )
                    # normalization: recip the replicated Z rows, DMA-move the
                    # reciprocal block to the values rows' partitions, multiply.
                    with nc.named_scope(f"norm_p{p}_c{c}"):
                        zr = z_p.tile([P, CH], f32, tag="zr")
                        zm = z_p.tile([P, CH], f32, tag="zm")
                        cs = slice(c * CH, (c + 1) * CH)
                        # even head: values rows 0:64, Z rows 64:128
                        nc.vector.reciprocal(out=zr[64:128, :], in_=psv[0][64:128, :])
                        nc.sync.dma_start(out=zm[0:64, :], in_=zr[64:128, :])
                        nc.vector.tensor_mul(
                            out=valT[p][0:64, cs], in0=psv[0][0:64, :],
                            in1=zm[0:64, :],
                        )
                        # odd head: Z rows 0:64, values rows 64:128
                        nc.vector.reciprocal(out=zr[0:64, :], in_=psv[1][0:64, :])
                        nc.sync.dma_start(out=zm[64:128, :], in_=zr[0:64, :])
                        nc.vector.tensor_mul(
                            out=valT[p][64:128, cs], in0=psv[1][64:128, :],
                            in1=zm[64:128, :],
                        )

            # ---------- output projection ----------
            def outproj():
                with nc.named_scope("outproj"):
                    for nt2 in range(MT // 2):
                        ob = wT_p.tile([P, 2, DSTR], f32, tag="outbuf")
                        for half in range(2):
                            nt = 2 * nt2 + half
                            for ds in range(DSTR // NB):
                                ps = ps_p.tile([P, NB], f32,
                                               tag="AB"[(2 * nt + ds) % 2])
                                for p in range(2):
                                    nc.tensor.matmul(
                                        ps,
                                        lhsT=valT[p][:, nt * P:(nt + 1) * P],
                                        rhs=wout_sb[:, p, ds * NB:(ds + 1) * NB],
                                        start=(p == 0), stop=(p == 1),
                                    )
                                obs = ob[:, half, ds * NB:(ds + 1) * NB]
                                if ds == 0:
                                    nc.scalar.copy(out=obs, in_=ps)
                                else:
                                    nc.vector.tensor_copy(out=obs, in_=ps)
                        eng = nc.sync if nt2 % 2 == 0 else nc.scalar
                        eng.dma_start(
                            out=out[nt2 * 2 * P:(nt2 + 1) * 2 * P, :].rearrange(
                                "(h p) d -> p h d", p=P),
                            in_=ob)

            proj_qk(0)
            proj_qk(2)
            proj_v()
            for c in range(NCH):
                attn(0, c)
            proj_qk(1)
            proj_qk(3)
            for c in range(NCH):
                attn(1, c)
            outproj()

    nc.compile()
    return nc


def get_nc():
    if "nc" not in _BUILT:
        _BUILT["nc"] = _build_nc()
    return _BUILT["nc"]


def _shard_inputs(stream, mask, w_qkv, b_qkv, w_out):
    """Build the 8 per-core input maps (host-side layout transforms)."""
    stream = np.asarray(stream, np.float32)
    mask = np.asarray(mask, np.float32)
    w_qkv = np.asarray(w_qkv, np.float32)
    b_qkv = np.asarray(b_qkv, np.float32)
    w_out = np.asarray(w_out, np.float32)

    emT = np.exp(mask[0].T).astype(BF16)  # (N, N) exp of transposed mask
    xT = [np.ascontiguousarray(stream[:, b, :].T) for b in range(B)]

    in_maps = []
    for d in range(N_CORES):
        b = d // 4
        heads = [(d % 4) * 4 + j for j in range(NH)]
        qc = [w_qkv[:, h * DHEAD:h * DHEAD + DKQ] for h in heads]
        kc = [w_qkv[:, h * DHEAD + DKQ:h * DHEAD + 2 * DKQ] for h in heads]
        vc = [w_qkv[:, h * DHEAD + 2 * DKQ:(h + 1) * DHEAD] for h in heads]
        wqk = np.ascontiguousarray(np.concatenate(
            [qc[0], qc[1], qc[2], qc[3], kc[0], kc[1], kc[2], kc[3]], axis=1))
        wv = np.ascontiguousarray(np.concatenate(vc, axis=1))
        bq = [b_qkv[h * DHEAD:h * DHEAD + DKQ] for h in heads]
        bk = [b_qkv[h * DHEAD + DKQ:h * DHEAD + 2 * DKQ] for h in heads]
        bvv = [b_qkv[h * DHEAD + 2 * DKQ:(h + 1) * DHEAD] for h in heads]
        bqk_arr = np.stack(
            [np.concatenate([bq[0], bq[1]]), np.concatenate([bq[2], bq[3]]),
             np.concatenate([bk[0], bk[1]]), np.concatenate([bk[2], bk[3]])],
            axis=1).astype(np.float32)
        bv_arr = np.ascontiguousarray(np.concatenate(bvv)[None, :])
        woutd = np.ascontiguousarray(
            np.concatenate([w_out[h * DV:(h + 1) * DV, :] for h in heads], axis=0))
        in_maps.append({
            "xT": xT[b], "wqk": wqk, "wv": wv, "bqk": bqk_arr, "bv": bv_arr,
            "ones": np.ones((1, P), np.float32), "emT": emT, "wout": woutd,
        })
    return in_maps


def kernel(stream, mask, w_qkv, b_qkv, w_out, b_out):
    nc = get_nc()
    in_maps = _shard_inputs(stream, mask, w_qkv, b_qkv, w_out)
    res = run_bass_kernel_spmd(nc, in_maps, core_ids=list(range(N_CORES)))
    b_out = np.asarray(b_out, np.float32)
    out = np.empty((N, B, DSTR), np.float32)
    for b in range(B):
        acc = res.results[4 * b]["out"].copy()
        for i in range(1, 4):
            acc += res.results[4 * b + i]["out"]
        out[:, b, :] = acc + b_out
    return out


# revision 32
# speedup vs baseline: 1.2769x; 1.0175x over previous
"""MultiHeadAttention Trainium2 kernel (8-core SPMD, head/tensor parallel).

Problem (hardcoded shapes): stream (2048, 2, 1024) f32, mask (1, 2048, 2048),
w_qkv (1024, 3072), b_qkv (3072,), w_out (1024, 1024), b_out (1024,).
N=2048, B=2, HEADS=16, D_KQ=D_V=64, D_HEAD=192.

Sharding (per the b*heads head-parallel hint): core d handles batch b=d//4 and
the 4 heads [4*(d%4), 4*(d%4)+4): w_qkv columns and w_out rows are split per
head group, logits/weights are fully local per core, and the post-projection
all-reduce (sum over the 4 cores of each batch, + b_out) is done on the host
during unsharding.

Per-core compute, all in "transposed" orientation so no on-device transposes
are needed (the host pre-transposes stream and mask while sharding):

  qkT[f, n]   = (w_qkv_local.T @ x_b.T)[f, n] + b     (f = head-pair d dims)
  v[m, dv]    = (x_b @ w_v_local)[m, dv] + b_v        (bias via a K=1 matmul)
  logitsT     = per head: lT[m, n] = sum_d k[m,d] q[n,d]   (2 heads row-packed
                in the PE array: K=64 each at tile_position (0,0)/(64,0))
  wT[m, n]    = exp(lT) * exp(maskT)[m, n]            (unnormalized softmax;
                exp(mask) is precomputed on the host, applied as a bf16
                multiply at DVE 2x rate; no max-subtraction needed at these
                logit magnitudes)
  psv         = PV matmul with lhsT = [v | ones-block] so the output carries
                values^T rows plus 64x-replicated row-sums Z (the softmax
                denominator) in the complementary partition half, for free
  valT[hv, n] = psv_values * recip(Z)                 (recip on DVE; the recip
                block is DMA-moved across partitions; DVE is lane-locked)
  out_partial = valT^T @ w_out_local   -> DMA to HBM, host sums per batch

dtypes: float32r (full-rate fp32) for the projections, QK^T and the output
projection; bf16 only on the exp->mask->PV path where DVE 2x mode needs it.
PSUM is managed as 4 rotating 2-bank slots (A/B for logits+projections+output,
V0/V1 for the two PV accumulators of the active head pair).
"""

import numpy as np
import ml_dtypes

import concourse.tile as tile
from concourse import bacc, mybir
from concourse.bass_utils import run_bass_kernel_spmd

BF16 = ml_dtypes.bfloat16
dt = mybir.dt
AF = mybir.ActivationFunctionType

# Shapes (hardcoded per the problem spec)
N = 2048          # sequence length
B = 2             # batch
DSTR = 1024       # d_stream
HEADS = 16        # total heads
NH = 4            # heads per core
DKQ = 64
DV = 64
DHEAD = 2 * DKQ + DV
P = 128
KT = DSTR // P    # 8 contraction k-tiles for projections
MT = N // P       # 16 m-tiles
CH = 1024         # attention n-chunk width
NCH = N // CH     # 2 chunks
NB = 512          # matmul moving free dim
N_CORES = 8

f32, f32r, bf16 = dt.float32, dt.float32r, dt.bfloat16

_BUILT = {}


def _build_nc():
    """Build + compile the single-core SPMD Bass program (same on all cores)."""
    nc = bacc.Bacc("TRN2", target_bir_lowering=False, debug=False)

    xT = nc.dram_tensor("xT", [DSTR, N], f32r, kind="ExternalInput").ap()
    wqk = nc.dram_tensor("wqk", [DSTR, 4 * P], f32r, kind="ExternalInput").ap()
    wv = nc.dram_tensor("wv", [DSTR, NH * DV], f32r, kind="ExternalInput").ap()
    bqk = nc.dram_tensor("bqk", [P, 4], f32, kind="ExternalInput").ap()
    bv = nc.dram_tensor("bv", [1, NH * DV], f32r, kind="ExternalInput").ap()
    ones = nc.dram_tensor("ones", [1, P], f32r, kind="ExternalInput").ap()
    emT = nc.dram_tensor("emT", [N, N], bf16, kind="ExternalInput").ap()
    wout = nc.dram_tensor("wout", [NH * DV, DSTR], f32r, kind="ExternalInput").ap()
    out = nc.dram_tensor("out", [N, DSTR], f32, kind="ExternalOutput").ap()

    with tile.TileContext(nc) as tc:
        with (
            tc.tile_pool(name="consts", bufs=1) as consts,
            tc.tile_pool(name="xw", bufs=1) as xw_p,
            tc.tile_pool(name="qkT", bufs=1) as qkT_p,
            tc.tile_pool(name="v1", bufs=1) as v1_p,
            tc.tile_pool(name="valT", bufs=1) as valT_p,
            tc.tile_pool(name="mask", bufs=3) as mask_p,
            tc.tile_pool(name="wT", bufs=3) as wT_p,
            tc.tile_pool(name="z", bufs=1) as z_p,
            tc.tile_pool(name="ps", bufs=1, space="PSUM") as ps_p,
        ):
            # ---------- persistent SBUF ----------
            xT_sb = xw_p.tile([P, KT, N], f32r)
            wqk_sb = xw_p.tile([P, KT, 4 * P], f32r)
            wv_sb = xw_p.tile([P, KT, NH * DV], f32r)
            bqk_sb = consts.tile([P, 4], f32)
            nc.scalar.dma_start(out=bqk_sb, in_=bqk)
            for kt in range(KT):
                e1 = nc.sync if kt % 2 == 0 else nc.scalar
                e2 = nc.scalar if kt % 2 == 0 else nc.sync
                e1.dma_start(out=xT_sb[:, kt, :], in_=xT[kt * P:(kt + 1) * P, :])
                e2.dma_start(out=wqk_sb[:, kt, :], in_=wqk[kt * P:(kt + 1) * P, :])
                e2.dma_start(out=wv_sb[:, kt, :], in_=wv[kt * P:(kt + 1) * P, :])
            bv_sb = consts.tile([1, NH * DV], f32r)
            nc.sync.dma_start(out=bv_sb, in_=bv)
            ones1 = consts.tile([1, P], f32r)
            nc.sync.dma_start(out=ones1, in_=ones)
            wout_sb = consts.tile([P, 2, DSTR], f32r)
            nc.sync.dma_start(
                out=wout_sb, in_=wout.rearrange("(i p) d -> p i d", p=P))

            # qkT f-tiles: 0 = q pair0, 1 = q pair1, 2 = k pair0, 3 = k pair1
            # (within a tile: partitions 0:64 = even head's d, 64:128 = odd head's)
            qkT = qkT_p.tile([P, 4, N], f32r)
            # v1 lhsT slots per (mt, head): even-in-pair = [v | ones],
            # odd-in-pair = [ones | v] -> PV output carries values rows and
            # 64x-replicated Z rows in complementary partition halves.
            v1 = v1_p.tile([P, MT, NH, P], bf16)
            nc.vector.memset(v1, 0.0)
            nc.vector.memset(v1[:, :, 0::2, 64:128], 1.0)
            nc.vector.memset(v1[:, :, 1::2, 0:64], 1.0)
            valT = [valT_p.tile([P, N], f32r, tag=f"valT{i}", name=f"valT{i}")
                    for i in range(2)]

            # ---------- projections ----------
            def proj_qk(ft):
                with nc.named_scope(f"proj_qk{ft}"):
                    for nb in range(N // NB):
                        ps = ps_p.tile([P, NB], f32, tag=["A", "B", "V0", "V1"][nb % 4])
                        for kt in range(KT):
                            nc.tensor.matmul(
                                ps,
                                lhsT=wqk_sb[:, kt, ft * P:(ft + 1) * P],
                                rhs=xT_sb[:, kt, nb * NB:(nb + 1) * NB],
                                start=(kt == 0), stop=(kt == KT - 1),
                            )
                        nc.scalar.activation(
                            out=qkT[:, ft, nb * NB:(nb + 1) * NB], in_=ps,
                            func=AF.Identity, bias=bqk_sb[:, ft:ft + 1],
                        )

            def proj_v():
                with nc.named_scope("proj_v"):
                    for mt in range(MT):
                        ps = ps_p.tile([P, NH * DV], f32, tag=["A", "B", "V0", "V1"][mt % 4])
                        for kt in range(KT):
                            nc.tensor.matmul(
                                ps,
                                lhsT=xT_sb[:, kt, mt * P:(mt + 1) * P],
                                rhs=wv_sb[:, kt, :],
                                start=(kt == 0), stop=False,
                            )
                        nc.tensor.matmul(
                            ps, lhsT=ones1, rhs=bv_sb,
                            start=False, stop=True,
                        )
                        psj = ps.rearrange("p (j d) -> p j d", d=DV)
                        nc.vector.tensor_copy(
                            out=v1[:, mt, 0::2, 0:DV], in_=psj[:, 0::2, :])
                        nc.vector.tensor_copy(
                            out=v1[:, mt, 1::2, 64:64 + DV], in_=psj[:, 1::2, :])

            # ---------- attention for one head pair over one n-chunk ----------
            def attn(p, c):
                with nc.named_scope(f"attn_p{p}_c{c}"):
                    psv = [
                        ps_p.tile([P, CH], f32, tag=f"V{oe}",
                                  name=f"psv{oe}_p{p}_c{c}")
                        for oe in (0, 1)
                    ]
                    for mt in range(MT):
                        em = mask_p.tile([P, CH], bf16)
                        nc.sync.dma_start(
                            out=em,
                            in_=emT[mt * P:(mt + 1) * P, c * CH:(c + 1) * CH],
                        )
                        for oe in (0, 1):
                            j = 2 * p + oe
                            base = oe * 64
                            psl = ps_p.tile([P, CH], f32, tag="AB"[oe])
                            for h2 in range(CH // NB):
                                nc.tensor.matmul(
                                    psl[:, h2 * NB:(h2 + 1) * NB],
                                    lhsT=qkT[base:base + 64, 2 + p,
                                                mt * P:(mt + 1) * P],
                                    rhs=qkT[base:base + 64, p,
                                               c * CH + h2 * NB:
                                               c * CH + (h2 + 1) * NB],
                                    start=True, stop=True,
                                )
                            wt = wT_p.tile([P, CH], bf16)
                            nc.scalar.activation(out=wt, in_=psl, func=AF.Exp)
                            nc.vector.tensor_mul(out=wt, in0=wt, in1=em)
                            for h2 in range(CH // NB):
                                nc.tensor.matmul(
                                    psv[oe][:, h2 * NB:(h2 + 1) * NB],
                                    lhsT=v1[:, mt, j, :],
                                    rhs=wt[:, h2 * NB:(h2 + 1) * NB],
                                    start=(mt == 0), stop=(mt == MT - 1),
                                )
                    # normalization: recip the replicated Z rows, DMA-move the
                    # reciprocal block to the values rows' partitions, multiply.
                    with nc.named_scope(f"norm_p{p}_c{c}"):
                        zr = z_p.tile([P, CH], f32, tag="zr")
                        zm = z_p.tile([P, CH], f32, tag="zm")
                        cs = slice(c * CH, (c + 1) * CH)
                        # even head: values rows 0:64, Z rows 64:128
                        nc.vector.reciprocal(out=zr[64:128, :], in_=psv[0][64:128, :])
                        nc.sync.dma_start(out=zm[0:64, :], in_=zr[64:128, :])
                        nc.vector.tensor_mul(
                            out=valT[p][0:64, cs], in0=psv[0][0:64, :],
                            in1=zm[0:64, :],
                        )
                        # odd head: Z rows 0:64, values rows 64:128
                        nc.vector.reciprocal(out=zr[0:64, :], in_=psv[1][0:64, :])
                        nc.sync.dma_start(out=zm[64:128, :], in_=zr[0:64, :])
                        nc.vector.tensor_mul(
                            out=valT[p][64:128, cs], in0=psv[1][64:128, :],
                            in1=zm[64:128, :],
                        )

            # ---------- output projection ----------
            def outproj():
                with nc.named_scope("outproj"):
                    for nt2 in range(MT // 2):
                        ob = wT_p.tile([P, 2, DSTR], f32, tag="outbuf")
                        for half in range(2):
                            nt = 2 * nt2 + half
                            for ds in range(DSTR // NB):
                                ps = ps_p.tile([P, NB], f32,
                                               tag="AB"[(2 * nt + ds) % 2])
                                for p in range(2):
                                    nc.tensor.matmul(
                                        ps,
                                        lhsT=valT[p][:, nt * P:(nt + 1) * P],
                                        rhs=wout_sb[:, p, ds * NB:(ds + 1) * NB],
                                        start=(p == 0), stop=(p == 1),
                                    )
                                obs = ob[:, half, ds * NB:(ds + 1) * NB]
                                if ds == 0:
                                    nc.scalar.copy(out=obs, in_=ps)
                                else:
                                    nc.vector.tensor_copy(out=obs, in_=ps)
                        eng = nc.sync if nt2 % 2 == 0 else nc.scalar
                        eng.dma_start(
                            out=out[nt2 * 2 * P:(nt2 + 1) * 2 * P, :].rearrange(
                                "(h p) d -> p h d", p=P),
                            in_=ob)

            proj_qk(0)
            proj_qk(2)
            proj_v()
            for c in range(NCH):
                attn(0, c)
            proj_qk(1)
            proj_qk(3)
            for c in range(NCH):
                attn(1, c)
            outproj()

    nc.compile()
    return nc


def get_nc():
    if "nc" not in _BUILT:
        _BUILT["nc"] = _build_nc()
    return _BUILT["nc"]


def _shard_inputs(stream, mask, w_qkv, b_qkv, w_out):
    """Build the 8 per-core input maps (host-side layout transforms)."""
    stream = np.asarray(stream, np.float32)
    mask = np.asarray(mask, np.float32)
    w_qkv = np.asarray(w_qkv, np.float32)
    b_qkv = np.asarray(b_qkv, np.float32)
    w_out = np.asarray(w_out, np.float32)

    emT = np.exp(mask[0].T).astype(BF16)  # (N, N) exp of transposed mask
    xT = [np.ascontiguousarray(stream[:, b, :].T) for b in range(B)]

    in_maps = []
    for d in range(N_CORES):
        b = d // 4
        heads = [(d % 4) * 4 + j for j in range(NH)]
        qc = [w_qkv[:, h * DHEAD:h * DHEAD + DKQ] for h in heads]
        kc = [w_qkv[:, h * DHEAD + DKQ:h * DHEAD + 2 * DKQ] for h in heads]
        vc = [w_qkv[:, h * DHEAD + 2 * DKQ:(h + 1) * DHEAD] for h in heads]
        wqk = np.ascontiguousarray(np.concatenate(
            [qc[0], qc[1], qc[2], qc[3], kc[0], kc[1], kc[2], kc[3]], axis=1))
        wv = np.ascontiguousarray(np.concatenate(vc, axis=1))
        bq = [b_qkv[h * DHEAD:h * DHEAD + DKQ] for h in heads]
        bk = [b_qkv[h * DHEAD + DKQ:h * DHEAD + 2 * DKQ] for h in heads]
        bvv = [b_qkv[h * DHEAD + 2 * DKQ:(h + 1) * DHEAD] for h in heads]
        bqk_arr = np.stack(
            [np.concatenate([bq[0], bq[1]]), np.concatenate([bq[2], bq[3]]),
             np.concatenate([bk[0], bk[1]]), np.concatenate([bk[2], bk[3]])],
            axis=1).astype(np.float32)
        bv_arr = np.ascontiguousarray(np.concatenate(bvv)[None, :])
        woutd = np.ascontiguousarray(
            np.concatenate([w_out[h * DV:(h + 1) * DV, :] for h in heads], axis=0))
        in_maps.append({
            "xT": xT[b], "wqk": wqk, "wv": wv, "bqk": bqk_arr, "bv": bv_arr,
            "ones": np.ones((1, P), np.float32), "emT": emT, "wout": woutd,
        })
    return in_maps


def kernel(stream, mask, w_qkv, b_qkv, w_out, b_out):
    nc = get_nc()
    in_maps = _shard_inputs(stream, mask, w_qkv, b_qkv, w_out)
    res = run_bass_kernel_spmd(nc, in_maps, core_ids=list(range(N_CORES)))
    b_out = np.asarray(b_out, np.float32)
    out = np.empty((N, B, DSTR), np.float32)
    for b in range(B):
        acc = res.results[4 * b]["out"].copy()
        for i in range(1, 4):
            acc += res.results[4 * b + i]["out"]
        out[:, b, :] = acc + b_out
    return out
